# revision 26
# baseline (speedup 1.0000x reference)
"""Multi-head self-attention on 8 trn2 NeuronCores.

Problem: B=4, S=2048, E=1024, H=8, D=128 MHA with a boolean attention mask.

Sharding: batch x head-group. Core c computes batch b=c//2 for heads
[4*(c%2), 4*(c%2)+4). Each core produces a partial output [S, E] (its 4
heads' contribution through w_out); the host sums the two partials per
batch. No on-device collectives needed.

v2 design: ONE software-pipelined stream. The attention unit stream
(head, pair, kt) is ACT/DVE-paced (exp + mask-mul + denominator adds),
so the PE has ~25% slack per unit; all other PE work (projections for
head h+1, V transposes, and the w_out contraction) is injected into
that stream as "filler" chunks so the PE never idles. The denominator
lives on the DVE (accD bf16 adds) except the final partition-reduce
(4 tiny selector matmuls per pair). PSUM: lg ring 3 half-banks + av 2
+ sum 2 + fill 1 = 8 banks exactly.

Perf history (HW exec): 379.0us -> ~300.3us (phase-split balanced
kernel, see git history) -> this restructure. Known dead ends: fp8
anywhere (error budget 2e-2, fp8 costs ~5% rms), GpSimd offload
(shares SBUF ports with DVE, 4x slowdown), bf16 PSUM matmul output
(TRN3-only, bass asserts fp32), full-tensor ACT bias (bias must be
per-partition scalar).
"""

import math
from collections import deque

import ml_dtypes
import numpy as np

import concourse.bass as bass
import concourse.tile as tile
from concourse import mybir
from concourse.bass_utils import run_bass_kernel_spmd
from concourse.masks import make_identity
from concourse.vector_clock import ScopedClock, VectorClock

B, S, E, H, D = 4, 2048, 1024, 8, 128
HPC = 4          # heads per core
NCORES = 8
NKT = S // 128   # key tiles per sequence
NET = E // 128   # contraction tiles for the projections
NQT = S // 128   # query tiles for the output projection
SCALE = 1.0 / math.sqrt(D)
BF16 = mybir.dt.bfloat16
F32 = mybir.dt.float32
F16 = mybir.dt.float16
EXP = mybir.ActivationFunctionType.Exp
LN = mybir.ActivationFunctionType.Ln

_patched = False


def _patch_drain():
    """The installed walrus rejects >1 sem wait on the Tile tail Drain.
    Emit one drain per pending logical processor instead."""
    global _patched
    if _patched:
        return
    _patched = True

    def _drain_and_barrier(self, tick_clock, wait_clock):
        nc = self.nc
        ticks = list(tick_clock.global_clock)
        procs = [i for i, t in enumerate(ticks) if t > 0]
        for p in procs or [None]:
            vec = [0] * len(ticks)
            if p is not None:
                vec[p] = ticks[p]
            d = nc.sync.drain()
            wait_clock.add_sem_waits(d.ins, ScopedClock({None: VectorClock(vec)}))
        nc.all_engine_barrier()
        popped = nc._tile_sem_poison_stack.pop()
        assert popped is self._sem_poison
        nc.clear_and_free_semaphores(list(self.sems.allocated().values()))
        nc.all_engine_barrier()

    tile.TileContext._drain_and_barrier = _drain_and_barrier


def _split_waits(nc):
    """This walrus build only encodes ONE sem wait per instruction. Move
    extra waits onto preceding same-engine NoOps (engines execute their
    instructions in block order, so this is semantically identical)."""
    import bass_rust

    k = 0
    for f in nc.m.functions:
        for bb in f.blocks:
            out = []
            for inst in bb.instructions:
                si = inst.sync_info
                if si is not None and si.on_wait and len(si.on_wait) > 1:
                    waits = list(si.on_wait)
                    for w in waits[:-1]:
                        nop = bass_rust.InstNoOp(
                            name=f"I-waitsplit-{k}", ins=[], outs=[]
                        )
                        k += 1
                        nop.engine = inst.engine
                        nop.sync_info = mybir.SyncInfo(on_wait=[w], on_update=[])
                        out.append(nop)
                    inst.sync_info = mybir.SyncInfo(
                        on_wait=[waits[-1]], on_update=si.on_update
                    )
                out.append(inst)
            bb.instructions[:] = out
    return nc


_nc_cache = None


def _build_nc():
    global _nc_cache
    if _nc_cache is not None:
        return _nc_cache
    _patch_drain()

    nc = bass.Bass()
    qT_d = nc.declare_dram_parameter("qT", [E, S], BF16, isOutput=False)
    keepT_d = nc.declare_dram_parameter("keepT", [S, S], BF16, isOutput=False)
    wq_d = nc.declare_dram_parameter("wq", [HPC, E, D], BF16, isOutput=False)
    wk_d = nc.declare_dram_parameter("wk", [HPC, E, D], BF16, isOutput=False)
    wv_d = nc.declare_dram_parameter("wv", [HPC, E, D], BF16, isOutput=False)
    wo_d = nc.declare_dram_parameter("wo", [HPC, D, E], BF16, isOutput=False)
    out_d = nc.declare_dram_parameter("out", [S, E], F16, isOutput=True)

    keepT_ap = keepT_d[:, :].rearrange("(kt p) q -> p kt q", p=128)
    qT_ap = qT_d[:, :].rearrange("(kt p) s -> p kt s", p=128)
    w_aps = {
        "wq": wq_d[:, :, :].rearrange("h (kt p) d -> p (h kt) d", p=128),
        "wk": wk_d[:, :, :].rearrange("h (kt p) d -> p (h kt) d", p=128),
        "wv": wv_d[:, :, :].rearrange("h (kt p) d -> p (h kt) d", p=128),
    }

    with tile.TileContext(nc) as tc:
        with (
            tc.tile_pool(name="const", bufs=1) as constp,
            tc.tile_pool(name="keep", bufs=1) as keepp,
            tc.tile_pool(name="qkv", bufs=1) as qkvp,
            tc.tile_pool(name="hT", bufs=1) as hTp,
            tc.tile_pool(name="wo", bufs=1) as wop,
            tc.tile_pool(name="qTd", bufs=1) as qTp,
            tc.tile_pool(name="wr", bufs=2) as wrp,
            tc.tile_pool(name="expt", bufs=4) as expp,
            tc.tile_pool(name="accs", bufs=2) as accp,
            tc.tile_pool(name="vt", bufs=2) as vtp,
            tc.tile_pool(name="small", bufs=2) as smallp,
            tc.tile_pool(name="avs", bufs=2) as avsp,
            tc.tile_pool(name="outs", bufs=3) as outsp,
            tc.tile_pool(name="ps_lg", bufs=1, space="PSUM") as ps_lgp,
            tc.tile_pool(name="ps_av", bufs=2, space="PSUM") as ps_av,
            tc.tile_pool(name="ps_sum", bufs=1, space="PSUM") as ps_sum,
            tc.tile_pool(name="ps_fill", bufs=1, space="PSUM") as ps_fill,
        ):
            # ---- constants ----
            zeros = constp.tile([128, 512], BF16)
            nc.vector.memset(zeros, 0.0)
            ident = constp.tile([128, 128], BF16)
            make_identity(nc, ident)
            # sums selectors: [128,2] lhsT writing the two 512-query halves
            # into rows 0/1 of one [2,512] PSUM tile
            sumsL_a = constp.tile([128, 2], BF16)
            nc.vector.memset(sumsL_a, 0.0)
            nc.vector.memset(sumsL_a[:, 0:1], 1.0)
            sumsL_b = constp.tile([128, 2], BF16)
            nc.vector.memset(sumsL_b, 0.0)
            nc.vector.memset(sumsL_b[:, 1:2], 1.0)
            # broadcast selectors: [2,128] lhsT replicating row 0/1 of rcb
            # onto all 128 partitions
            bcL_a = constp.tile([2, 128], BF16)
            nc.vector.memset(bcL_a, 0.0)
            nc.vector.memset(bcL_a[0:1, :], 1.0)
            bcL_b = constp.tile([2, 128], BF16)
            nc.vector.memset(bcL_b, 1.0)
            nc.vector.memset(bcL_b[0:1, :], 0.0)
            dumm = constp.tile([2, 32], F32)
            nc.vector.memset(dumm, 1.0)

            # ---- persistent SBUF ----
            keep_s = keepp.tile([128, NKT, S], BF16)
            QT_a = [qkvp.tile([128, S], BF16, tag=f"QT{h}", name=f"QT{h}") for h in range(HPC)]
            KT_a = [qkvp.tile([128, S], BF16, tag=f"KT{h}", name=f"KT{h}") for h in range(HPC)]
            V_a = [qkvp.tile([128, NKT, 128], BF16, tag=f"V{h}", name=f"V{h}") for h in range(HPC)]
            headsT_s = hTp.tile([128, HPC, S], BF16)
            wo_s = wop.tile([128, HPC, E], BF16)
            qT_s = qTp.tile([128, NET, S], BF16)

            # per-head weight staging ring (2 heads deep): [wq|wk|wv]
            w_tiles = {}

            def w_tile(h):
                if h not in w_tiles:
                    w_tiles[h] = wrp.tile(
                        [128, 3, NET, D], BF16, tag="w", name=f"w{h}"
                    )
                return w_tiles[h]

            WIDX = {"wq": 0, "wk": 1, "wv": 2}

            def load_w(h, names=("wq", "wk", "wv"), eng=None):
                eng = eng or nc.sync
                wt = w_tile(h)
                for nm in names:
                    eng.dma_start(
                        out=wt[:, WIDX[nm], :, :],
                        in_=w_aps[nm][:, h * NET : (h + 1) * NET, :],
                    )

            # ---------------- startup DMAs ----------------
            # DMA issue order tracks prologue consumption order: wq, then
            # the qT half-slices each chunk reads, with wk/wv slotted in
            # before their chunks. keep/wo/w(h1) ride behind.
            nc.sync.dma_start(
                out=w_tile(0)[:, 0, 0:1, :], in_=w_aps["wq"][:, 0:1, :]
            )
            nc.scalar.dma_start(out=qT_s[:, 0, 0:512], in_=qT_ap[:, 0, 0:512])
            nc.sync.dma_start(
                out=w_tile(0)[:, 0, 1:NET, :], in_=w_aps["wq"][:, 1:NET, :]
            )
            for kt in range(1, NET):
                eng = (nc.sync, nc.scalar)[kt % 2]
                eng.dma_start(out=qT_s[:, kt, 0:512], in_=qT_ap[:, kt, 0:512])
            load_w(0, ("wk",), eng=nc.sync)
            for kt in range(NET):
                eng = (nc.sync, nc.scalar)[kt % 2]
                eng.dma_start(out=qT_s[:, kt, 512:1024], in_=qT_ap[:, kt, 512:1024])
            load_w(0, ("wv",), eng=nc.scalar)
            for half in range(2):
                for kt in range(NET):
                    eng = (nc.sync, nc.scalar)[(kt + half) % 2]
                    lo = 1024 + half * 512
                    eng.dma_start(
                        out=qT_s[:, kt, lo : lo + 512], in_=qT_ap[:, kt, lo : lo + 512]
                    )
            load_w(1)
            # keep pair-0 halves (needed from unit 0 at ~40us)
            for kt in range(NKT):
                eng = (nc.sync, nc.scalar)[kt % 2]
                eng.dma_start(
                    out=keep_s[:, kt, 0:1024], in_=keepT_ap[:, kt, 0:1024]
                )
            nc.sync.dma_start(
                out=wo_s, in_=wo_d[:, :, :].rearrange("h d e -> d h e")
            )

            # preload the natural_log_exp ACT table set (one-time ~2.7us)
            nc.scalar.activation(dumm, dumm, LN)
            nc.scalar.activation(dumm, dumm, EXP)

            # HAM warmup: keep the PE busy from ~10us so the clock gate
            # opens before the real matmuls arrive
            wfill = ps_fill.tile([128, 512], F32, tag="fill", name="wfill")
            for _ in range(10):
                nc.tensor.matmul(
                    wfill, lhsT=zeros[:, 0:128], rhs=zeros, start=True, stop=True
                )

            # ---------------- filler chunk machinery ----------------
            # One chunk ~= 1.7us of PE work + an ACT/DVE evacuation.
            vt_cur = [None]

            def proj_chunk(h, wname, st2, half, po_of=None):
                def emit():
                    q0 = st2 * 1024 + half * 512
                    wt = w_tile(h)
                    if po_of is not None:
                        po = po_of()
                    else:
                        po = ps_fill.tile([128, 512], F32, tag="fill", name="po")
                    for et in range(NET):
                        nc.tensor.matmul(
                            po,
                            lhsT=wt[:, WIDX[wname], et, :],
                            rhs=qT_s[:, et, q0 : q0 + 512],
                            start=(et == 0),
                            stop=(et == NET - 1),
                        )
                    if wname == "wv":
                        if half == 0:
                            vt_cur[0] = vtp.tile([128, 1024], BF16, tag="vt", name="vt")
                        nc.scalar.copy(
                            vt_cur[0][:, half * 512 : (half + 1) * 512], po
                        )
                    else:
                        dst = QT_a[h] if wname == "wq" else KT_a[h]
                        nc.vector.tensor_copy(dst[:, q0 : q0 + 512], po)

                return emit

            def transpose_chunk(h, st2):
                def emit():
                    vt = vt_cur[0]
                    pst = ps_fill.tile([128, 8, 128], BF16, tag="fill", name="pst")
                    for j in range(8):
                        nc.tensor.transpose(
                            pst[:, j, :], vt[:, j * 128 : (j + 1) * 128], ident
                        )
                    nc.vector.tensor_copy(
                        V_a[h][:, st2 * 8 : (st2 + 1) * 8, :], pst
                    )

                return emit

            def head_chunks(h):
                ck = []
                for st2 in range(2):
                    for wname in ("wq", "wk", "wv"):
                        for half in range(2):
                            ck.append(proj_chunk(h, wname, st2, half))
                    ck.append(transpose_chunk(h, st2))
                return ck

            def ph3_chunk(qt, half, po_of=None):
                def emit():
                    if po_of is not None:
                        po = po_of()
                    else:
                        po = ps_fill.tile([128, 512], F32, tag="fill", name="po3")
                    for h in range(HPC):
                        nc.tensor.matmul(
                            po,
                            lhsT=headsT_s[:, h, qt * 128 : (qt + 1) * 128],
                            rhs=wo_s[:, h, half * 512 : (half + 1) * 512],
                            start=(h == 0),
                            stop=(h == HPC - 1),
                        )
                    ob = outsp.tile([128, 512], F16, tag="ob", name="ob")
                    if half == 0:
                        nc.scalar.copy(ob, po)
                    else:
                        nc.vector.tensor_copy(ob, po)
                    # out DMAs stay on the sync queue: a DMA instruction
                    # waiting at the scalar queue head blocks ALL later
                    # activations (observed 46us tail from exactly this)
                    nc.sync.dma_start(
                        out=out_d[
                            qt * 128 : (qt + 1) * 128, half * 512 : (half + 1) * 512
                        ],
                        in_=ob,
                    )

                return emit

            # lg ring: one flat [128, 2048] f32 tile = 4 PSUM banks, manual
            # half-slot rotation. Unit u's halves land in slots (2u)%4 and
            # (2u+1)%4 — always bank-adjacent, always disjoint from unit
            # u+1's slots, so the exp is one op and the one-unit logits
            # lookahead never overwrites a slot the current exp still reads.
            # Outside the attention stream (prologue/tail) these banks are
            # idle, so dense PE-chunk sequences rotate through them too.
            lg_s = ps_lgp.tile([128, 2048], F32)
            rcnt = [0]

            def rot_po():
                i = rcnt[0]
                rcnt[0] += 1
                if i % 5 < 4:
                    return lg_s[:, (i % 5) * 512 : (i % 5 + 1) * 512]
                return ps_fill.tile([128, 512], F32, tag="fill", name="po")

            # ---------------- proj(h0): dense prologue ----------------
            # st2=0 chunks first (their qT tiles land first)
            for st2 in range(2):
                for wname in ("wq", "wk", "wv"):
                    for half in range(2):
                        proj_chunk(0, wname, st2, half, rot_po)()
                transpose_chunk(0, st2)()

            # w ring slot 0 is free again only after h0's chunks above, so
            # the h2 load must be emitted after them (ring reuse order);
            # w slot 1 (h3) frees only after h1's filler chunks.
            load_w(2)
            for kt in range(NKT):
                nc.sync.dma_start(
                    out=keep_s[:, kt, 1024:2048],
                    in_=keepT_ap[:, kt, 1024:2048],
                )

            # filler queue: (mark_in_units, emit_fn)
            fillers = deque()
            for h in range(1, HPC):
                ck = head_chunks(h)
                w0 = 32 * (h - 1)
                for i, c in enumerate(ck):
                    fillers.append((w0 + i * 30.0 / len(ck), c))
                if h == 1:
                    fillers.append((30.5, lambda: load_w(3)))
            # ph3 for pair 0: only after (h3,p0)'s normalize is EMITTED
            # (unit 113, kt==1 finish block) — marks start at 113.2
            for i, qt in enumerate(range(8)):
                fillers.append((113.2 + i * 1.8, ph3_chunk(qt, 0)))
                fillers.append((113.2 + i * 1.8 + 0.9, ph3_chunk(qt, 1)))
            fillers = deque(sorted(fillers, key=lambda t: t[0]))

            def pump(u):
                while fillers and fillers[0][0] <= u:
                    fillers.popleft()[1]()

            # ---------------- attention stream ----------------

            def unit(u):
                return u // 32, (u // 16) % 2, u % 16

            def emit_lg(u):
                h, pair, kt = unit(u)
                q0 = pair * 1024
                for c in range(2):
                    s = ((2 * u + c) % 4) * 512
                    nc.tensor.matmul(
                        lg_s[:, s : s + 512],
                        lhsT=KT_a[h][:, kt * 128 : (kt + 1) * 128],
                        rhs=QT_a[h][:, q0 + c * 512 : q0 + (c + 1) * 512],
                        start=True,
                        stop=True,
                    )

            def emit_exp(u, ex):
                s0 = ((2 * u) % 4) * 512
                nc.scalar.activation(
                    ex, lg_s[:, s0 : s0 + 1024], EXP, scale=SCALE
                )

            # deferred normalization chain (one pair deep). The two halves
            # finish one unit apart (kt==1 and kt==2) so sm2/pb0/pb1 can
            # serialize through a SINGLE psum bank without stalling the PE.
            pending = None

            def _finish_half(item, half):
                avs0, avs1, rcb, ph, pq0 = item
                pb = ps_sum.tile([128, 512], F32, tag="ps_sum", name="pb")
                nc.tensor.matmul(
                    pb, lhsT=(bcL_a, bcL_b)[half], rhs=rcb, start=True, stop=True
                )
                return pb

            def _finish_half_dve(item, half, pb):
                avs0, avs1, rcb, ph, pq0 = item
                src = (avs0, avs1)[half]
                off = pq0 + half * 512
                nc.vector.tensor_mul(headsT_s[:, ph, off : off + 512], src, pb)

            NU = HPC * 2 * NKT
            av0 = av1 = accD = exD_first = None
            haveD = False
            emit_lg(0)
            for u in range(NU):
                h, pair, kt = unit(u)
                q0 = pair * 1024
                if kt == 0:
                    av0 = ps_av.tile([128, 512], F32, tag="ps_av", name="av0")
                    av1 = ps_av.tile([128, 512], F32, tag="ps_av", name="av1")
                    accD = accp.tile([128, 1024], BF16, tag="accD", name="accD")
                    exD_first = None
                    haveD = False
                if u + 1 < NU:
                    emit_lg(u + 1)
                if kt in (1, 2) and pending is not None:
                    pb = _finish_half(pending, kt - 1)
                ex = expp.tile([128, 1024], BF16, tag="ex", name="ex")
                emit_exp(u, ex)
                nc.vector.tensor_mul(ex, ex, keep_s[:, kt, q0 : q0 + 1024])
                if kt in (1, 2) and pending is not None:
                    _finish_half_dve(pending, kt - 1, pb)
                    if kt == 2:
                        pending = None
                # denominator: DVE accumulates kt 0..14; kt 15 joins on the
                # PE during the fold (its keep-mul would stall the fold)
                if kt < NKT - 1:
                    if exD_first is None:
                        exD_first = ex
                    elif not haveD:
                        nc.vector.tensor_add(accD, exD_first, ex)
                        haveD = True
                    else:
                        nc.vector.tensor_add(accD, accD, ex)
                # filler BEFORE the av matmuls: av waits on the DVE
                # keep-multiply, so the PE chews filler during that window
                pump(u)
                first, last = kt == 0, kt == NKT - 1
                nc.tensor.matmul(
                    av0, lhsT=V_a[h][:, kt, :], rhs=ex[:, 0:512],
                    start=first, stop=last,
                )
                nc.tensor.matmul(
                    av1, lhsT=V_a[h][:, kt, :], rhs=ex[:, 512:1024],
                    start=first, stop=last,
                )
                if kt == NKT - 1:
                    # fold: accD (kt 0..14) + ex15, partition-reduced onto
                    # rows 0/1 of a [2,512] PSUM tile
                    sm2 = ps_sum.tile([2, 512], F32, tag="ps_sum", name="sm2")
                    nc.tensor.matmul(
                        sm2, lhsT=sumsL_a, rhs=accD[:, 0:512],
                        start=True, stop=False,
                    )
                    nc.tensor.matmul(
                        sm2, lhsT=sumsL_b, rhs=accD[:, 512:1024],
                        start=False, stop=False,
                    )
                    nc.tensor.matmul(
                        sm2, lhsT=sumsL_a, rhs=ex[:, 0:512],
                        start=False, stop=False,
                    )
                    nc.tensor.matmul(
                        sm2, lhsT=sumsL_b, rhs=ex[:, 512:1024],
                        start=False, stop=True,
                    )
                    # reciprocal via ln + exp(-x) (both in natural_log_exp)
                    lnsm = smallp.tile(
                        [2, 512], F32, tag="lnsm", name="lnsm", bufs=1
                    )
                    nc.scalar.activation(lnsm, sm2, LN)
                    rcb = smallp.tile([2, 512], BF16, tag="rcb", name="rcb")
                    nc.scalar.activation(rcb, lnsm, EXP, scale=-1.0)
                    # evacuate accumulators promptly (frees PSUM banks)
                    avs0 = avsp.tile([128, 512], BF16, tag="avs", name="avs0")
                    avs1 = avsp.tile([128, 512], BF16, tag="avs", name="avs1")
                    nc.scalar.copy(avs0, av0)
                    nc.vector.tensor_copy(avs1, av1)
                    pending = (avs0, avs1, rcb, h, q0)
            if pending is not None:
                for half in range(2):
                    pb = _finish_half(pending, half)
                    _finish_half_dve(pending, half, pb)
                pending = None

            # ---------------- tail: remaining output projection ----------
            while fillers:
                fillers.popleft()[1]()
            # rotate the tail chunks through the (now idle) lg banks + the
            # fill bank so evacuations overlap the next chunk's matmuls
            for qt in range(8, NQT):
                ph3_chunk(qt, 0, rot_po)()
                ph3_chunk(qt, 1, rot_po)()

    _split_waits(nc)
    _nc_cache = nc
    return nc


def kernel(q, mask, w_query, w_key, w_value, w_out):
    nc = _build_nc()
    bf16 = ml_dtypes.bfloat16

    qT = np.ascontiguousarray(np.transpose(q.astype(bf16), (0, 2, 1)))
    keepT = np.ascontiguousarray(np.transpose((~mask).astype(bf16), (0, 2, 1)))
    wq = np.ascontiguousarray(w_query.astype(bf16))
    wk = np.ascontiguousarray(w_key.astype(bf16))
    wv = np.ascontiguousarray(w_value.astype(bf16))
    wo = np.ascontiguousarray(w_out.astype(bf16))

    in_maps = []
    for c in range(NCORES):
        b, g = c // 2, c % 2
        hs = slice(g * HPC, (g + 1) * HPC)
        in_maps.append(
            {
                "qT": qT[b],
                "keepT": keepT[b],
                "wq": wq[hs],
                "wk": wk[hs],
                "wv": wv[hs],
                "wo": wo[hs],
            }
        )

    global _last_in_maps
    _last_in_maps = in_maps
    res = run_bass_kernel_spmd(nc, in_maps, list(range(NCORES)))
    outs = [r["out"].astype(np.float32) for r in res.results]
    return np.stack([outs[2 * b] + outs[2 * b + 1] for b in range(B)])
